# revision 35
# baseline (speedup 1.0000x reference)
"""Local (sliding-window) MQA attention block on 8 Trainium2 NeuronCores, v2.

Sharding: data-parallel over batch (4) x sequence-parallel over query halves
(2) = 8 cores. Each core computes 1024 query rows of one batch against a
2048-row key halo (window=1024), all 16 query heads, single shared KV head.

v2 vs v1: all matmul operands in bf16 (f32 PSUM accumulate), which halves
DMA/SBUF and lets the whole kernel run as one pass over all 16 heads:
  A) k/v projections over the halo; the query-half x chunks land in a
     persistent SBUF tile (xq) that phase B reuses — x is loaded once.
  B) q projection in 8 waves of 2 heads (4 PSUM banks/wave, two waves in
     flight), weights double-buffered on the gpsimd DMA queue; RoPE fused
     into the PSUM->SBUF eviction.
  C) attention per (query-block, head): S^T matmuls, exp on ACT (halo
     padding via per-partition bias), structural causal/window masks as
     post-exp 0/1 multiplies on DVE (bf16, 2x mode), denominator via
     ones-matmul PSUM chain, PV accumulation, fast-reciprocal normalize.
  D) output projection accumulated over heads + bias, weights prefetched
     on the sync queue during C.
"""
import sys

for _p in ("/opt/trn_rl_repo",):
    if _p not in sys.path:
        sys.path.insert(0, _p)

import numpy as np
import ml_dtypes

import concourse.bass as bass
import concourse.bacc as bacc
import concourse.tile as tile
import concourse.mybir as mybir
from concourse.bass_utils import run_bass_kernel_spmd

F32 = mybir.dt.float32
BF16 = mybir.dt.bfloat16
EXP = mybir.ActivationFunctionType.Exp

B, T, W = 4, 2048, 2048
NH, HD = 16, 128
WIN = 1024
QL = 1024          # query rows per core
KB = 2048          # key-halo rows per core
QBS = 512          # query block (moving free dim)
NQB = QL // QBS    # 2 query blocks per core
SLOTS = (WIN + QBS) // 128  # 12 key slots of 128 per query block
NEG = -1.0e9
SCALE = HD ** -0.5
MAX_WAVELENGTH = 10000.0
NW = W // 128      # 16 width chunks

_COMPILED = None


def _rope_tables(pos):
    """pos: [n] int -> (cmul, smul) [128, n] f32 such that
    rope(x)[d] = x[d]*cmul[d] + x[shuf(d)]*smul[d], shuf(d)=d^32 for d<64."""
    half = 32
    inv_freq = MAX_WAVELENGTH ** (-(2.0 * np.arange(half, dtype=np.float64)) / 64.0)
    ang = pos.astype(np.float64)[None, :] * inv_freq[:, None]   # [32, n]
    sin, cos = np.sin(ang), np.cos(ang)
    n = pos.shape[0]
    cmul = np.ones((HD, n), dtype=np.float64)
    smul = np.zeros((HD, n), dtype=np.float64)
    cmul[0:32] = cos
    cmul[32:64] = cos
    smul[0:32] = -sin
    smul[32:64] = sin
    return cmul.astype(np.float32), smul.astype(np.float32)


def _emit_rope(nc, pool, dst, src_ps, cmul, smul, n):
    """dst[0:64] = src[0:64]*c[0:64] + shuf(src)[0:64]*s[0:64]; dst[64:128]=src.
    dst: SBUF AP [128, n] (any dtype); src_ps: PSUM f32 AP; cmul/smul SBUF f32.
    Partition-shuffle + pass-through copies run on ScalarE to offload DVE."""
    t1 = pool.tile([64, n], F32, tag="rope_t1", bufs=2)
    t2 = pool.tile([64, n], F32, tag="rope_t2", bufs=2)
    stage = pool.tile([64, n], F32, tag="rope_stage", bufs=2)
    nc.vector.tensor_mul(t1[:, :], src_ps[0:64, :], cmul[0:64, :])
    nc.scalar.copy(out=stage[0:32, :], in_=src_ps[32:64, :])
    nc.scalar.copy(out=stage[32:64, :], in_=src_ps[0:32, :])
    nc.vector.tensor_mul(t2[:, :], stage[:, :], smul[0:64, :])
    nc.vector.tensor_add(dst[0:64, :], t1[:, :], t2[:, :])
    nc.scalar.copy(out=dst[64:128, :], in_=src_ps[64:128, :])


def _build_program():
    nc = bacc.Bacc("TRN2", target_bir_lowering=False, debug=False)

    # weights are host-packed partition-major so every DMA line is one
    # long contiguous descriptor per partition (cheap enqueue + full BW)
    xtb = nc.dram_tensor("xtb", [W, KB], BF16, kind="ExternalInput")
    wqtb = nc.dram_tensor("wqtb", [128, 8, NW * 256], BF16,
                          kind="ExternalInput")
    wkb = nc.dram_tensor("wkb", [128, NW * HD], BF16, kind="ExternalInput")
    wvb = nc.dram_tensor("wvb", [128, NW * HD], BF16, kind="ExternalInput")
    wotb = nc.dram_tensor("wotb", [128, 4, NW * 512], BF16,
                          kind="ExternalInput")
    bias = nc.dram_tensor("bias", [W], F32, kind="ExternalInput")
    cq_d = nc.dram_tensor("cq", [HD, QL], BF16, kind="ExternalInput")
    sq_d = nc.dram_tensor("sq", [HD, QL], BF16, kind="ExternalInput")
    ck_d = nc.dram_tensor("ck", [HD, KB], BF16, kind="ExternalInput")
    sk_d = nc.dram_tensor("sk", [HD, KB], BF16, kind="ExternalInput")
    maskb = nc.dram_tensor("maskb", [NQB, 8, 128, QBS], BF16,
                           kind="ExternalInput")
    identb_d = nc.dram_tensor("identb", [128, 128], BF16, kind="ExternalInput")
    onesb_d = nc.dram_tensor("onesb", [128, 1], BF16, kind="ExternalInput")
    padb_d = nc.dram_tensor("padb", [128, KB // 128], F32, kind="ExternalInput")
    out = nc.dram_tensor("out", [QL, W], F32, kind="ExternalOutput")

    with tile.TileContext(nc) as tc:
        with tc.tile_pool(name="persist", bufs=1) as pp:
            encT = pp.tile([HD, NH, QL], BF16, tag="encT")   # per-head enc^T
            qT = pp.tile([HD, NH, QL], BF16, tag="qT")       # rope'd q^T
            xq = pp.tile([128, NW, QL], BF16, tag="xq")      # query-half x^T
            kT = pp.tile([HD, KB], BF16, tag="kT")           # rope'd k^T
            v_sb = pp.tile([128, KB], BF16, tag="v")         # natural v
            cq = pp.tile([HD, QL], BF16, tag="cq")
            sq = pp.tile([HD, QL], BF16, tag="sq")
            ones_sb = pp.tile([128, 1], BF16, tag="ones")
            ident = pp.tile([128, 128], BF16, tag="ident")
            bias_bc = pp.tile([128, W], F32, tag="biasbc")
            padb = pp.tile([128, KB // 128], F32, tag="padb")

            nc.gpsimd.dma_start(out=ones_sb[:, :], in_=onesb_d[:, :])
            nc.gpsimd.dma_start(out=ident[:, :], in_=identb_d[:, :])
            nc.gpsimd.dma_start(out=cq[:, :], in_=cq_d[:, :])
            nc.gpsimd.dma_start(out=sq[:, :], in_=sq_d[:, :])
            nc.gpsimd.dma_start(out=padb[:, :], in_=padb_d[:, :])
            b_ap = bias.ap()
            nc.gpsimd.dma_start(out=bias_bc[:, :], in_=bass.AP(
                tensor=b_ap.tensor, offset=b_ap.offset,
                ap=[[0, 128]] + list(b_ap.ap)))

            # wq-weight double buffer lives below phase A's transients so
            # its prefetch DMAs never WAR-wait on phase A reads.
            pbw_stack = tc.tile_pool(name="pbw", bufs=1)
            pbw = pbw_stack.__enter__()

            # ---------- Phase A: k/v projections over the halo ----------
            with tc.tile_pool(name="pa", bufs=3) as pa, \
                 tc.tile_pool(name="pa1", bufs=1) as pa1, \
                 tc.tile_pool(name="pa_ps", bufs=2, space="PSUM") as paps:
                ck = pa1.tile([HD, KB], BF16, tag="ck")
                sk = pa1.tile([HD, KB], BF16, tag="sk")
                nc.gpsimd.dma_start(out=ck[:, :], in_=ck_d[:, :])
                nc.gpsimd.dma_start(out=sk[:, :], in_=sk_d[:, :])
                wk_sb = pa1.tile([128, NW, HD], BF16, tag="wk")
                wv_sb = pa1.tile([128, NW, HD], BF16, tag="wv")
                nc.scalar.dma_start(out=wk_sb[:, :, :], in_=wkb[:, :])
                nc.scalar.dma_start(out=wv_sb[:, :, :], in_=wvb[:, :])
                # window-half chunks stream on sync, query-half (persistent
                # xq) on scalar — two queues feed the PE in parallel.
                xta = [pa.tile([128, 1024], BF16, tag="xtA", bufs=4,
                               name="xtA") for _ in range(NW)]
                for wc in range(NW):
                    nc.sync.dma_start(
                        out=xta[wc][:, :],
                        in_=xtb[128 * wc:128 * (wc + 1), 0:1024])
                for wc in range(NW):
                    nc.scalar.dma_start(
                        out=xq[:, wc, :],
                        in_=xtb[128 * wc:128 * (wc + 1), 1024:2048])
                for sq2 in range(2):
                    kt_ps = [paps.tile([HD, 512], F32, tag="kt_ps",
                                       name="kt_ps", bufs=3) for _ in range(2)]
                    vt_ps = [paps.tile([HD, 512], F32, tag="vt_ps",
                                       name="vt_ps", bufs=3) for _ in range(2)]
                    for wc in range(NW):
                        if sq2 == 0:
                            xv = xta[wc][:, :]
                        else:
                            xv = xq[:, wc, :]
                        for hf in range(2):
                            nc.tensor.matmul(
                                out=kt_ps[hf][:, :], lhsT=wk_sb[:, wc, :],
                                rhs=xv[:, QBS * hf:QBS * (hf + 1)],
                                start=(wc == 0), stop=(wc == NW - 1))
                            nc.tensor.matmul(
                                out=vt_ps[hf][:, :], lhsT=wv_sb[:, wc, :],
                                rhs=xv[:, QBS * hf:QBS * (hf + 1)],
                                start=(wc == 0), stop=(wc == NW - 1))
                    for hf in range(2):
                        sq4 = 2 * sq2 + hf
                        cols = slice(512 * sq4, 512 * (sq4 + 1))
                        _emit_rope(nc, pa, kT[:, cols], kt_ps[hf][:, :],
                                   ck[:, cols], sk[:, cols], 512)
                        # v: copy PSUM->SBUF bf16 then PE-transpose 128-blocks
                        vt_sb = pa.tile([HD, 512], BF16, tag="vt_sb")
                        nc.vector.tensor_copy(out=vt_sb[:, :],
                                              in_=vt_ps[hf][:, :])
                        for j in range(4):
                            vps2 = paps.tile([128, 128], BF16, tag="vT2")
                            nc.tensor.transpose(
                                vps2[:, :], vt_sb[:, 128 * j:128 * (j + 1)],
                                ident[:, :])
                            blk = 4 * sq4 + j
                            nc.vector.tensor_copy(
                                out=v_sb[:, 128 * blk:128 * (blk + 1)],
                                in_=vps2[:, :])

            # wot + mask prefetch buffers: allocated over phase A's (dead)
            # transient region, loaded during phase B, consumed in C/D.
            pdw_stack = tc.tile_pool(name="pdw", bufs=1)
            pdw = pdw_stack.__enter__()
            wot_sbs = [pdw.tile([128, NW, 512], BF16, tag="wot", name="wot",
                                bufs=2) for _ in range(4)]
            for oc in range(4):
                nc.sync.dma_start(out=wot_sbs[oc][:, :, :],
                                  in_=wotb[:, oc, :])
            masks_all = pdw.tile([128, NQB, 8, QBS], BF16, tag="masks")

            # ---------- Phase B: q projection, 8 waves of 2 heads ----------
            with tc.tile_pool(name="pb", bufs=3) as pb, \
                 tc.tile_pool(name="pb_ps", bufs=8, space="PSUM") as pbps:
                for wave in range(8):
                    h0 = 2 * wave
                    q_ps = [[pbps.tile([HD, QBS], F32, tag="q_ps",
                                       name="q_ps")
                             for _ in range(2)] for _ in range(2)]
                    wq_w = pbw.tile([128, NW, 256], BF16, tag="wqw", bufs=2)
                    nc.gpsimd.dma_start(out=wq_w[:, :, :],
                                        in_=wqtb[:, wave, :])
                    for wc in range(NW):
                        for hj in range(2):
                            for qh in range(2):
                                nc.tensor.matmul(
                                    out=q_ps[hj][qh][:, :],
                                    lhsT=wq_w[:, wc,
                                              128 * hj:128 * (hj + 1)],
                                    rhs=xq[:, wc, QBS * qh:QBS * (qh + 1)],
                                    start=(wc == 0),
                                    stop=(wc == NW - 1))
                    for hj in range(2):
                        for qh in range(2):
                            _emit_rope(
                                nc, pb,
                                qT[:, h0 + hj, QBS * qh:QBS * (qh + 1)],
                                q_ps[hj][qh][:, :],
                                cq[:, QBS * qh:QBS * (qh + 1)],
                                sq[:, QBS * qh:QBS * (qh + 1)], QBS)

            # mask loads sit on the gpsimd queue AFTER the wq prefetches so
            # they can't head-of-line-block phase B's weight pipeline.
            for i in range(NQB):
                for mc in range(8):
                    nc.gpsimd.dma_start(out=masks_all[:, i, mc, :],
                                        in_=maskb[i, mc, :, :])

            # ---------- Phase C: attention ----------
            # Slots 0-3 carry the window left-edge mask, 8-11 the causal
            # mask, applied as post-exp 0/1 multiplies on DVE; 4-7 are
            # mask-free (halo padding handled by the exp bias + v=0).
            MASKED = (0, 1, 2, 3, 8, 9, 10, 11)
            MCOL = {k: (k if k < 4 else k - 4) for k in MASKED}
            GS = 2  # slots per pipeline group
            NG = SLOTS // GS
            with tc.tile_pool(name="pc", bufs=2) as pc, \
                 tc.tile_pool(name="et", bufs=8) as pe_t, \
                 tc.tile_pool(name="pc_s", bufs=6, space="PSUM") as pcs, \
                 tc.tile_pool(name="pc_e", bufs=1, space="PSUM") as pce, \
                 tc.tile_pool(name="pc_d", bufs=1, space="PSUM") as pcd:
                for i in range(NQB):
                    for head in range(NH):
                        enc_ps = pce.tile([HD, QBS], F32, tag="enc_ps")
                        den_ps = pcd.tile([1, QBS], F32, tag="den_ps")
                        ets = [None] * SLOTS
                        eps = [None] * NG
                        qs = qT[:, head, QBS * i:QBS * (i + 1)]

                        def emit_s_group(g):
                            sps = []
                            for kk in range(GS):
                                k = GS * g + kk
                                s_ps = pcs.tile([128, QBS], F32, tag="s_ps")
                                c0 = 512 * i + 128 * k
                                nc.tensor.matmul(
                                    out=s_ps[:, :],
                                    lhsT=kT[:, c0:c0 + 128],
                                    rhs=qs, start=True, stop=True)
                                sps.append(s_ps)
                            for kk in range(GS):
                                k = GS * g + kk
                                blk = 4 * i + k
                                et = pe_t.tile([128, QBS], BF16, tag="et")
                                if k in MCOL:
                                    e0 = pc.tile([128, QBS], BF16, tag="e0",
                                                 bufs=4)
                                    nc.scalar.activation(
                                        out=e0[:, :], in_=sps[kk][:, :],
                                        func=EXP,
                                        bias=padb[:, blk:blk + 1])
                                    nc.vector.tensor_mul(
                                        et[:, :], e0[:, :],
                                        masks_all[:, i, MCOL[k], :])
                                else:
                                    nc.scalar.activation(
                                        out=et[:, :], in_=sps[kk][:, :],
                                        func=EXP,
                                        bias=padb[:, blk:blk + 1])
                                ets[k] = et
                            # pair-sum on DVE halves the PE den chain
                            ep = pc.tile([128, QBS], BF16, tag="ep", bufs=4)
                            nc.vector.tensor_add(
                                ep[:, :], ets[GS * g][:, :],
                                ets[GS * g + 1][:, :])
                            eps[g] = ep

                        def emit_acc_group(g):
                            for kk in range(GS):
                                k = GS * g + kk
                                blk = 4 * i + k
                                nc.tensor.matmul(
                                    out=enc_ps[:, :],
                                    lhsT=v_sb[:, 128 * blk:128 * (blk + 1)],
                                    rhs=ets[k][:, :],
                                    start=(k == 0),
                                    stop=(k == SLOTS - 1))
                            nc.tensor.matmul(
                                out=den_ps[:, :],
                                lhsT=ones_sb[:, :],
                                rhs=eps[g][:, :],
                                start=(g == 0),
                                stop=(g == NG - 1))

                        # software pipeline: S three groups ahead of acc
                        LA = 3
                        for g in range(LA):
                            emit_s_group(g)
                        for g in range(LA, NG):
                            emit_s_group(g)
                            emit_acc_group(g - LA)
                        for g in range(NG - LA, NG):
                            emit_acc_group(g)

                        den_sb = pc.tile([1, QBS], F32, tag="den_sb")
                        nc.vector.reciprocal_approx_fast(
                            den_sb[:, :], den_ps[:, :])
                        den_bc = pc.tile([128, QBS], F32, tag="den_bc")
                        nc.gpsimd.partition_broadcast(
                            den_bc[:, :], den_sb[:, :])
                        nc.vector.tensor_mul(
                            encT[:, head, QBS * i:QBS * (i + 1)],
                            enc_ps[:, :], den_bc[:, :])

            # ---------- Phase D: output projection ----------
            with tc.tile_pool(name="pdo", bufs=3) as pdo, \
                 tc.tile_pool(name="pd_ps", bufs=3, space="PSUM") as pdps:
                for oc in range(4):
                    wot_sb = wot_sbs[oc]
                    for tsub in range(QL // 128):
                        o_ps = pdps.tile([128, 512], F32, tag="o_ps")
                        for n in range(NH):
                            nc.tensor.matmul(
                                out=o_ps[:, :],
                                lhsT=encT[:, n, 128 * tsub:128 * (tsub + 1)],
                                rhs=wot_sb[:, n, :],
                                start=(n == 0), stop=(n == NH - 1))
                        o_sb = pdo.tile([128, 512], F32, tag="o_sb")
                        nc.vector.tensor_add(o_sb[:, :], o_ps[:, :],
                                             bias_bc[:, 512 * oc:512 * (oc + 1)])
                        nc.sync.dma_start(
                            out=out[128 * tsub:128 * (tsub + 1),
                                    512 * oc:512 * (oc + 1)],
                            in_=o_sb[:, :])

            pdw_stack.__exit__(None, None, None)
            pbw_stack.__exit__(None, None, None)

    nc.compile()
    return nc


def _get_program():
    global _COMPILED
    if _COMPILED is None:
        _COMPILED = _build_program()
    return _COMPILED


def _prep_core_inputs(x, segment_pos, attention_mask, shared):
    """Per-core input dicts. Core c: batch c//2, query half c%2."""
    segment_pos = np.asarray(segment_pos)
    attention_mask = np.asarray(attention_mask)
    in_maps = []
    for c in range(8):
        b, h = c // 2, c % 2
        key_start = QL * h - WIN
        # halo buffer rows [key_start, key_start + KB) of batch b, zero-padded
        kb = np.zeros((KB, W), dtype=np.float32)
        lo = max(0, -key_start)
        kb[lo:] = x[b, key_start + lo:key_start + KB]
        xtb = np.ascontiguousarray(kb.T).astype(ml_dtypes.bfloat16)

        g_q = QL * h + np.arange(QL)                      # global query rows
        g_k = key_start + np.arange(KB)                   # global key rows
        pos_q = segment_pos[g_q]
        pos_k = np.where((g_k >= 0) & (g_k < T), segment_pos[np.clip(g_k, 0, T - 1)], 0)
        cq, sq = _rope_tables(pos_q)
        ck, sk = _rope_tables(pos_k)

        # 0/1 multiplicative mask per (query block i, masked slot) in S^T
        # layout [ds, dt]. Out-of-range (halo padding) rows keep 1 here:
        # they are excluded via the exp bias (padb) and contribute v=0.
        ma = np.ones((NQB, 8, 128, QBS), dtype=np.float32)
        MASKED = (0, 1, 2, 3, 8, 9, 10, 11)
        MCOL = {k: (k if k < 4 else k - 4) for k in MASKED}
        interior_bad = False
        for i in range(NQB):
            t_glob = g_q[QBS * i:QBS * (i + 1)]           # [dt=512]
            for k in range(SLOTS):
                r = QBS * i + 128 * k + np.arange(128)    # halo rows [ds]
                s_glob = key_start + r
                ok = (s_glob >= 0) & (s_glob < T)
                m = attention_mask[t_glob[None, :].repeat(128, 0),
                                   np.clip(s_glob, 0, T - 1)[:, None]]
                bad = ok[:, None] & ~m
                if k in MCOL:
                    ma[i, MCOL[k]][bad] = 0.0
                elif bad.any():
                    interior_bad = True
        if interior_bad:
            raise ValueError(
                "attention_mask penalizes interior window slots; this "
                "kernel assumes slots 4-7 are mask-free")
        ok_k = (g_k >= 0) & (g_k < T)
        padb = np.ascontiguousarray(np.where(
            ok_k, 0.0, NEG).astype(np.float32).reshape(KB // 128, 128).T)
        bf = ml_dtypes.bfloat16
        in_maps.append(dict(
            shared, xtb=xtb, cq=cq.astype(bf), sq=sq.astype(bf),
            ck=ck.astype(bf), sk=sk.astype(bf),
            maskb=ma.astype(bf), padb=padb))
    return in_maps


def _check_mask_coverage(attention_mask):
    """Every True entry for core-c queries must fall inside its 12 slots."""
    am = np.asarray(attention_mask)
    t = np.arange(T)[:, None]
    s = np.arange(T)[None, :]
    h = (t >= QL).astype(np.int64)
    key_start = QL * h - WIN
    i = ((t - QL * h) // QBS)
    lo = key_start + QBS * i
    covered = (s >= lo) & (s < lo + SLOTS * 128)
    if (am & ~covered).any():
        raise ValueError(
            "attention_mask has True entries outside the sliding-window "
            "block structure this kernel is specialized for")


def kernel(x, segment_pos, attention_mask, wq, wk, wv, w_out, b_out):
    x = np.asarray(x, dtype=np.float32)
    wq = np.asarray(wq, dtype=np.float32)
    wk = np.asarray(wk, dtype=np.float32)
    wv = np.asarray(wv, dtype=np.float32)
    w_out = np.asarray(w_out, dtype=np.float32)
    b_out = np.asarray(b_out, dtype=np.float32)

    _check_mask_coverage(attention_mask)

    nc = _get_program()
    bf = ml_dtypes.bfloat16

    def packw(wt, ngrp, gcols):
        # [W, ngrp*gcols] -> [128, ngrp, NW*gcols]: partition-major with one
        # contiguous line per (partition, group) for cheap DMA descriptors
        a = wt.reshape(NW, 128, ngrp, gcols).transpose(1, 2, 0, 3)
        return np.ascontiguousarray(a.reshape(128, ngrp, NW * gcols))

    wqt = np.ascontiguousarray(wq.T) * np.float32(SCALE)
    shared = {
        "wqtb": packw(wqt.astype(bf), 8, 256),
        "wkb": packw(wk.T.astype(bf), 1, HD)[:, 0],
        "wvb": packw(wv.T.astype(bf), 1, HD)[:, 0],
        "wotb": packw(w_out.T.astype(bf), 4, 512),
        "bias": b_out,
        "identb": np.eye(128, dtype=np.float32).astype(bf),
        "onesb": np.ones((128, 1), dtype=np.float32).astype(bf),
    }
    in_maps = _prep_core_inputs(x, segment_pos, attention_mask, shared)
    res = run_bass_kernel_spmd(nc, in_maps, list(range(8)))
    global _LAST_RESULT
    _LAST_RESULT = res

    out = np.empty((B, T, W), dtype=np.float32)
    for c in range(8):
        b, h = c // 2, c % 2
        out[b, QL * h:QL * (h + 1), :] = res.results[c]["out"]
    return out
